# revision 4
# baseline (speedup 1.0000x reference)
"""Trainium2 Bass kernel for the sparse-attention decoder problem.

Math (per batch b):
  fixed_context = mean_n(emb) @ W_context                       [H]
  K|V|LK        = emb @ W_kvlogit (split in 3)                  [N,H] each
  query         = fixed_context + [gather(emb,cur)|feat3] @ W_step
  per head h:   compat = (Q_h K_h^T)/8 ; softmax over masked N
  heads_out     = attn @ V_h ; glimpse = heads @ W_out
  logits        = tanh(glimpse LK^T / sqrt(H)) * 10 ; mask ; log_softmax

Device layout strategy (everything "T" = feature-major so the PE
contraction dim sits on SBUF partitions):
  embT  [D,N]  ->  KT=[e,N], V=[N,e], LKT=[o,N] via one pass of matmuls
  queryT=[H,T] = W_step^T nnT + fc ;   compat=[T,N] per head
  attnT via PE "transpose" = matmul(P_chunk^T @ diag(1/s)) -> [N,T]
  headsT=[h,T] = V^T attnT ;  gT=[o,T]=W_out^T headsT ; logits=gT^T LKT
Softmax skips the max-subtraction (compat is O(1); masked lanes get -1e8
added, exp underflows to exactly 0).  log_softmax likewise: z=10*tanh+ma,
out = z - log(sum(exp(z))).

Sharding: pure data-parallel over batch, 32 batches per core on 8 cores.
All matmul operands are bf16 (fp32 PSUM accumulation); host-simulated
end-to-end error vs the fp32 reference is ~2e-3 abs / 3.5e-4 rel on
unmasked outputs.
"""

import numpy as np
import ml_dtypes
from contextlib import ExitStack

import concourse.bass as bass
import concourse.tile as tile
from concourse import bacc, masks, mybir
from concourse.bass_utils import run_bass_kernel_spmd

B, N, D, H, HEADS, KEY, T = 256, 512, 512, 512, 8, 64, 128
NCORES = 8
BL = B // NCORES          # batches per core
DC = D // 128             # 4 d-chunks
KC = 5                    # padded D+3 -> 640 rows for the step projection
MA = -1e8                 # additive mask (underflows exp to 0)
F32 = mybir.dt.float32
BF16 = mybir.dt.bfloat16
AX = mybir.AxisListType.X
OP = mybir.AluOpType
AF = mybir.ActivationFunctionType

LAST_EXEC_TIME_NS = None


def _emit(ctx, tc, io, bl):
    nc = tc.nc
    embT, nnT, ma, wkv, wstep, wout, wctx, outp = io

    wp = ctx.enter_context(tc.tile_pool(name="wp", bufs=1))
    # weights, loaded once
    wkv_t = []
    for k in range(DC):
        wkv_k = wp.tile([128, 3 * H], BF16, name=f"wkv{k}")
        nc.sync.dma_start(wkv_k[:], wkv[k])
        wkv_t.append(wkv_k)
    wstep_t = []
    for k in range(KC):
        ws_k = wp.tile([128, H], BF16, name=f"wstep{k}")
        nc.sync.dma_start(ws_k[:], wstep[k])
        wstep_t.append(ws_k)
    wout_t = []
    for k in range(DC):
        wo_k = wp.tile([128, H], BF16, name=f"wout{k}")
        nc.sync.dma_start(wo_k[:], wout[k])
        wout_t.append(wo_k)
    wctx_t = []
    for k in range(DC):
        wc_k = wp.tile([128, H], BF16, name=f"wctx{k}")
        nc.sync.dma_start(wc_k[:], wctx[k])
        wctx_t.append(wc_k)
    ident = wp.tile([128, 128], BF16, name="ident")
    masks.make_identity(nc, ident[:])

    sb = ctx.enter_context(tc.tile_pool(name="sb", bufs=1))
    ps512 = ctx.enter_context(tc.tile_pool(name="ps512", bufs=4, space="PSUM"))
    psat = ctx.enter_context(tc.tile_pool(name="psat", bufs=2, space="PSUM"))
    pssm = ctx.enter_context(tc.tile_pool(name="pssm", bufs=2, space="PSUM"))

    for p in range(bl // 2):
        bs = (2 * p, 2 * p + 1)
        # ---- per-batch: embT load, K/V/LK projections, mean ----
        mt = sb.tile([128, DC * 2], F32, tag="mt", bufs=3, name=f"mt_{p}")
        kt_sb, v_sb, lk_sb, ma_sb = {}, {}, {}, {}
        for j, b in enumerate(bs):
            et = []
            for c in range(DC):
                e_c = sb.tile([128, N], BF16, tag="et", bufs=10, name=f"et{b}_{c}")
                nc.sync.dma_start(e_c[:], embT[b, c])
                et.append(e_c)
            for c in range(DC):
                nc.vector.reduce_sum(mt[:, 2 * c + j : 2 * c + j + 1], et[c][:], axis=AX)
            for m in range(DC):
                kt_ps = ps512.tile([128, N], F32, tag="ps512", name=f"ktps{b}{m}")
                for k in range(DC):
                    nc.tensor.matmul(kt_ps[:], wkv_t[k][:, bass.ts(m, 128)], et[k][:],
                                     start=(k == 0), stop=(k == DC - 1))
                kt = sb.tile([128, N], BF16, tag="kt", bufs=10, name=f"kt{b}_{m}")
                nc.vector.tensor_copy(kt[:], kt_ps[:])
                kt_sb[b, m] = kt
            for m in range(DC):
                v_ps = ps512.tile([128, H], F32, tag="ps512", name=f"vps{b}{m}")
                for k in range(DC):
                    nc.tensor.matmul(v_ps[:], et[k][:, bass.ts(m, 128)],
                                     wkv_t[k][:, H : 2 * H],
                                     start=(k == 0), stop=(k == DC - 1))
                v = sb.tile([128, H], BF16, tag="v", bufs=10, name=f"v{b}_{m}")
                nc.vector.tensor_copy(v[:], v_ps[:])
                v_sb[b, m] = v
            for m in range(DC):
                lk_ps = ps512.tile([128, N], F32, tag="ps512", name=f"lkps{b}{m}")
                for k in range(DC):
                    nc.tensor.matmul(lk_ps[:], wkv_t[k][:, 2 * H + m * 128 : 2 * H + (m + 1) * 128],
                                     et[k][:], start=(k == 0), stop=(k == DC - 1))
                lk = sb.tile([128, N], BF16, tag="lk", bufs=10, name=f"lk{b}_{m}")
                nc.vector.tensor_copy(lk[:], lk_ps[:])
                lk_sb[b, m] = lk
            ma_t = sb.tile([128, N], F32, tag="ma", bufs=4, name=f"ma{b}")
            nc.sync.dma_start(ma_t[:], ma[b])
            ma_sb[b] = ma_t

        # ---- per-pair: fixed context + query ----
        mtb = sb.tile([128, DC * 2], BF16, tag="mtb", bufs=3, name=f"mtb_{p}")
        nc.vector.tensor_copy(mtb[:], mt[:])
        fc_sb = []
        for m in range(DC):
            fc_ps = pssm.tile([128, 2], F32, tag="sm", name=f"fcps{p}{m}")
            for k in range(DC):
                nc.tensor.matmul(fc_ps[:], wctx_t[k][:, bass.ts(m, 128)],
                                 mtb[:, 2 * k : 2 * k + 2],
                                 start=(k == 0), stop=(k == DC - 1))
            fc_m = sb.tile([128, 2], F32, tag="fc", bufs=8, name=f"fc{p}_{m}")
            nc.scalar.activation(fc_m[:], fc_ps[:], AF.Copy, scale=0.125 / N)
            fc_sb.append(fc_m)
        nnt = []
        for k in range(KC):
            nn_k = sb.tile([128, 2 * T], BF16, tag="nnt", bufs=12, name=f"nnt{p}_{k}")
            nc.sync.dma_start(nn_k[:, 0:T], nnT[bs[0], k])
            nc.sync.dma_start(nn_k[:, T : 2 * T], nnT[bs[1], k])
            nnt.append(nn_k)
        qt_sb = []
        for m in range(DC):
            q_ps = pssm.tile([128, 2 * T], F32, tag="sm", name=f"qps{p}{m}")
            for k in range(KC):
                nc.tensor.matmul(q_ps[:], wstep_t[k][:, bass.ts(m, 128)], nnt[k][:],
                                 start=(k == 0), stop=(k == KC - 1))
            qt = sb.tile([128, 2 * T], BF16, tag="qt", bufs=6, name=f"qt{p}_{m}")
            for j in range(2):
                nc.scalar.activation(qt[:, j * T : (j + 1) * T], q_ps[:, j * T : (j + 1) * T],
                                     AF.Identity, bias=fc_sb[m][:, j : j + 1], scale=0.125)
            qt_sb.append(qt)

        # ---- per-batch: compat + softmax (unnormalized P + diag(1/s)) ----
        P_sb, dg_sb = {}, {}
        for j, b in enumerate(bs):
            for h in range(HEADS):
                m, o = h // 2, (h % 2) * 64
                cm_ps = ps512.tile([128, N], F32, tag="ps512", name=f"cm{b}{h}")
                nc.tensor.matmul(cm_ps[:],
                                 qt_sb[m][o : o + 64, j * T : (j + 1) * T],
                                 kt_sb[b, m][o : o + 64, :], start=True, stop=True)
                pm = sb.tile([128, N], F32, tag="pm", bufs=4, name=f"pm{b}{h}")
                nc.vector.tensor_tensor(pm[:], cm_ps[:], ma_sb[b][:], op=OP.add)
                pe = sb.tile([128, N], BF16, tag="pe", bufs=20, name=f"pe{b}{h}")
                s = sb.tile([128, 1], F32, tag="s", bufs=20, name=f"s{b}{h}")
                nc.scalar.activation(pe[:], pm[:], AF.Exp, accum_out=s[:])
                r = sb.tile([128, 1], F32, tag="r", bufs=20, name=f"r{b}{h}")
                nc.vector.reciprocal(r[:], s[:])
                dg = sb.tile([128, 128], BF16, tag="dg", bufs=20, name=f"dg{b}{h}")
                nc.vector.tensor_scalar_mul(dg[:], ident[:], r[:])
                P_sb[b, h] = pe
                dg_sb[b, h] = dg

        # ---- per-pair: transpose attn, heads_out ----
        hd_sb = [sb.tile([128, 2 * T], BF16, tag="hd", bufs=6, name=f"hd{p}_{m}")
                 for m in range(DC)]
        for h in range(HEADS):
            att = []
            for c in range(DC):
                at_ps = psat.tile([128, 2 * T], F32, tag="at", name=f"at{p}{h}{c}")
                for j, b in enumerate(bs):
                    nc.tensor.matmul(at_ps[:, j * T : (j + 1) * T],
                                     P_sb[b, h][:, bass.ts(c, 128)], dg_sb[b, h][:],
                                     start=True, stop=True)
                a_c = sb.tile([128, 2 * T], BF16, tag="att", bufs=6, name=f"att{p}{h}{c}")
                nc.vector.tensor_copy(a_c[:], at_ps[:])
                att.append(a_c)
            hd_ps = pssm.tile([64, 2 * T], F32, tag="sm", name=f"hdps{p}{h}")
            for j, b in enumerate(bs):
                for c in range(DC):
                    nc.tensor.matmul(hd_ps[:, j * T : (j + 1) * T],
                                     v_sb[b, c][:, h * 64 : (h + 1) * 64],
                                     att[c][:, j * T : (j + 1) * T],
                                     start=(c == 0), stop=(c == DC - 1))
            m, o = h // 2, (h % 2) * 64
            nc.vector.tensor_copy(hd_sb[m][o : o + 64, :], hd_ps[:])

        # ---- per-pair: glimpse ----
        g_sb = []
        for m in range(DC):
            g_ps = pssm.tile([128, 2 * T], F32, tag="sm", name=f"gps{p}{m}")
            for k in range(DC):
                nc.tensor.matmul(g_ps[:], wout_t[k][:, bass.ts(m, 128)], hd_sb[k][:],
                                 start=(k == 0), stop=(k == DC - 1))
            g_m = sb.tile([128, 2 * T], BF16, tag="g", bufs=6, name=f"g{p}_{m}")
            nc.vector.tensor_copy(g_m[:], g_ps[:])
            g_sb.append(g_m)

        # ---- per-batch: logits + log_softmax ----
        for j, b in enumerate(bs):
            lg_ps = ps512.tile([128, N], F32, tag="ps512", name=f"lg{b}")
            for k in range(DC):
                nc.tensor.matmul(lg_ps[:], g_sb[k][:, j * T : (j + 1) * T],
                                 lk_sb[b, k][:], start=(k == 0), stop=(k == DC - 1))
            y = sb.tile([128, N], F32, tag="y", bufs=3, name=f"y{b}")
            nc.scalar.activation(y[:], lg_ps[:], AF.Tanh, scale=float(1.0 / np.sqrt(H)))
            t2 = sb.tile([128, N], F32, tag="t2", bufs=3, name=f"t2{b}")
            nc.vector.tensor_tensor(t2[:], y[:], ma_sb[b][:], op=OP.add)
            p2 = sb.tile([128, N], BF16, tag="p2", bufs=2, name=f"p2{b}")
            s2 = sb.tile([128, 1], F32, tag="s2", bufs=4, name=f"s2{b}")
            nc.scalar.activation(p2[:], t2[:], AF.Exp, scale=10.0, accum_out=s2[:])
            lns = sb.tile([128, 1], F32, tag="lns", bufs=4, name=f"lns{b}")
            nc.scalar.activation(lns[:], s2[:], AF.Ln)
            o_t = sb.tile([128, N], F32, tag="o", bufs=3, name=f"o{b}")
            nc.vector.tensor_scalar(o_t[:], t2[:], 10.0, lns[:], op0=OP.mult, op1=OP.subtract)
            nc.sync.dma_start(outp[b], o_t[:])


def _build(bl):
    nc = bacc.Bacc("TRN2", target_bir_lowering=False, debug=False)
    embT = nc.dram_tensor("embT", [bl, DC, 128, N], BF16, kind="ExternalInput").ap()
    nnT = nc.dram_tensor("nnT", [bl, KC, 128, T], BF16, kind="ExternalInput").ap()
    ma = nc.dram_tensor("ma", [bl, T, N], F32, kind="ExternalInput").ap()
    wkv = nc.dram_tensor("wkv", [DC, 128, 3 * H], BF16, kind="ExternalInput").ap()
    wstep = nc.dram_tensor("wstep", [KC, 128, H], BF16, kind="ExternalInput").ap()
    wout = nc.dram_tensor("wout", [DC, 128, H], BF16, kind="ExternalInput").ap()
    wctx = nc.dram_tensor("wctx", [DC, 128, H], BF16, kind="ExternalInput").ap()
    outp = nc.dram_tensor("logp", [bl, T, N], F32, kind="ExternalOutput").ap()
    with tile.TileContext(nc) as tc, ExitStack() as ctx:
        _emit(ctx, tc, (embT, nnT, ma, wkv, wstep, wout, wctx, outp), bl)
    nc.compile()
    return nc


_cache = {}


def _program(bl):
    if bl not in _cache:
        _cache[bl] = _build(bl)
    return _cache[bl]


def _prep(embedding, current_nodes, used_capacity, used_battery, current_time, mask):
    bf = ml_dtypes.bfloat16
    b = embedding.shape[0]
    embT = np.ascontiguousarray(embedding.transpose(0, 2, 1)).astype(bf)
    embT = embT.reshape(b, DC, 128, N)
    cur = np.take_along_axis(embedding, current_nodes.astype(np.int64)[:, :, None], axis=1)
    nnT = np.zeros((b, KC * 128, T), np.float32)
    nnT[:, :D, :] = cur.transpose(0, 2, 1)
    nnT[:, D, :] = 1.0 - used_capacity
    nnT[:, D + 1, :] = 1.0 - used_battery
    nnT[:, D + 2, :] = current_time
    nnT = nnT.astype(bf).reshape(b, KC, 128, T)
    ma = np.where(mask, np.float32(MA), np.float32(0.0))
    return embT, nnT, ma


def _prep_weights(W_context, W_kvlogit, W_step, W_out):
    bf = ml_dtypes.bfloat16
    wkv = W_kvlogit.astype(bf).reshape(DC, 128, 3 * H)
    ws = np.zeros((KC * 128, H), np.float32)
    ws[: D + 3] = W_step
    wstep = ws.astype(bf).reshape(KC, 128, H)
    wout = W_out.astype(bf).reshape(DC, 128, H)
    wctx = W_context.astype(bf).reshape(DC, 128, H)
    return wkv, wstep, wout, wctx


def kernel(embedding, current_nodes, used_capacity, used_battery, current_time,
           mask, W_context, W_kvlogit, W_step, W_out):
    global LAST_EXEC_TIME_NS
    embedding = np.asarray(embedding, np.float32)
    mask = np.asarray(mask, bool)
    embT, nnT, ma = _prep(np.asarray(embedding, np.float32),
                          np.asarray(current_nodes),
                          np.asarray(used_capacity, np.float32),
                          np.asarray(used_battery, np.float32),
                          np.asarray(current_time, np.float32), mask)
    wkv, wstep, wout, wctx = _prep_weights(
        np.asarray(W_context, np.float32), np.asarray(W_kvlogit, np.float32),
        np.asarray(W_step, np.float32), np.asarray(W_out, np.float32))
    nc = _program(BL)
    in_maps = []
    for c in range(NCORES):
        s = slice(c * BL, (c + 1) * BL)
        in_maps.append({"embT": embT[s], "nnT": nnT[s], "ma": ma[s],
                        "wkv": wkv, "wstep": wstep, "wout": wout, "wctx": wctx})
    res = run_bass_kernel_spmd(nc, in_maps, list(range(NCORES)))
    LAST_EXEC_TIME_NS = res.exec_time_ns
    return np.concatenate([res.results[c]["logp"] for c in range(NCORES)], axis=0)


# revision 12
# speedup vs baseline: 13045.9222x; 13045.9222x over previous
"""Trainium2 Bass kernel for the sparse-attention decoder problem.

Math (per batch b):
  fixed_context = mean_n(emb) @ W_context                       [H]
  K|V|LK        = emb @ W_kvlogit (split in 3)                  [N,H] each
  query         = fixed_context + [gather(emb,cur)|feat3] @ W_step
  per head h:   compat = (Q_h K_h^T)/8 ; softmax over masked N
  heads_out     = attn @ V_h ; glimpse = heads @ W_out
  logits        = tanh(glimpse LK^T / sqrt(H)) * 10 ; mask ; log_softmax

Device layout strategy (everything "T" = feature-major so the PE
contraction dim sits on SBUF partitions):
  embT  [D,N]  ->  KT=[e,N], V=[N,e], LKT=[o,N] via one pass of matmuls
  queryT=[H,T] = W_step^T nnT + fc ;   compat=[T,N] per head
  attnT via PE matmul  P_chunk^T @ diag(1/s)  -> normalized [N,T]
  headsT=[h,T] = V^T attnT ;  gT=[o,T]=W_out^T headsT ; logits=gT^T LKT

Tricks:
  - masking: compat PSUM banks are seeded with identity@mask (one cheap
    bf16 matmul, start=True) so exp reads pre-masked PSUM directly; the
    -1e8 additive mask underflows exp to exactly 0.
  - softmax skips max-subtraction (unmasked compat is O(1)).
  - per-row sums come free from ACT activation accum_out; the graph mean
    likewise from an ACT copy with accum_out.
  - log_softmax: out = z - ln(sum exp z); ln is computed with a Mitchell
    bit-trick seed + 2 Newton steps so the ACT engine never leaves the
    exp/tanh/copy table set (no per-batch table reloads).
  - compat QK pairs use disjoint PE row groups, heads_out pairs disjoint
    col groups, so paired matmuls can overlap on the 128x128 array.

Sharding: pure data-parallel over batch, 32 batches per core on 8 cores.
All matmul operands are bf16 (fp32 PSUM accumulation); end-to-end error
vs the fp32 reference is ~2e-3 abs / 3.4e-4 rel on unmasked outputs.
Cost-model estimate ~690 us/core; measured on hardware ~750 us.
"""

import os
import numpy as np
import ml_dtypes
from contextlib import ExitStack

# the axon client in this image has no NTFF hook; a stray BASS_TRACE=1
# would crash run_bass_kernel_spmd, so pin tracing off for the exec path.
os.environ.setdefault("BASS_NEVER_TRACE", "1")

import concourse.bass as bass
import concourse.tile as tile
from concourse import bacc, masks, mybir
from concourse.bass_utils import run_bass_kernel_spmd

B, N, D, H, HEADS, KEY, T = 256, 512, 512, 512, 8, 64, 128
NCORES = 8
BL = B // NCORES          # batches per core
DC = D // 128             # 4 d-chunks
KC = 5                    # padded D+3 -> 640 rows for the step projection
MA = -1e8                 # additive mask (underflows exp to 0)
F32 = mybir.dt.float32
BF16 = mybir.dt.bfloat16
OP = mybir.AluOpType
AF = mybir.ActivationFunctionType

LAST_EXEC_TIME_NS = None


def _emit(ctx, tc, io, bl):
    nc = tc.nc
    embT, nnT, ma, mab, wkv, wstep, wout, wctx, outp = io

    wp = ctx.enter_context(tc.tile_pool(name="wp", bufs=1))
    # weights, loaded once
    wkv_t = []
    for k in range(DC):
        wkv_k = wp.tile([128, 3 * H], BF16, name=f"wkv{k}")
        nc.sync.dma_start(wkv_k[:], wkv[k])
        wkv_t.append(wkv_k)
    wstep_t = []
    for k in range(KC):
        ws_k = wp.tile([128, H], BF16, name=f"wstep{k}")
        nc.sync.dma_start(ws_k[:], wstep[k])
        wstep_t.append(ws_k)
    wout_t = []
    for k in range(DC):
        wo_k = wp.tile([128, H], BF16, name=f"wout{k}")
        nc.sync.dma_start(wo_k[:], wout[k])
        wout_t.append(wo_k)
    wctx_t = []
    for k in range(DC):
        wc_k = wp.tile([128, H], BF16, name=f"wctx{k}")
        nc.sync.dma_start(wc_k[:], wctx[k])
        wctx_t.append(wc_k)
    ident = wp.tile([128, 128], BF16, name="ident")
    masks.make_identity(nc, ident[:])

    sb = ctx.enter_context(tc.tile_pool(name="sb", bufs=1))
    pskv = ctx.enter_context(tc.tile_pool(name="pskv", bufs=3, space="PSUM"))
    pscm = ctx.enter_context(tc.tile_pool(name="pscm", bufs=2, space="PSUM"))
    pssm = ctx.enter_context(tc.tile_pool(name="pssm", bufs=3, space="PSUM"))

    for p in range(bl // 2):
        bs = (2 * p, 2 * p + 1)
        # ---- per-batch: embT load, K/V/LK projections, mean ----
        mt = sb.tile([128, DC * 2], F32, tag="mt", bufs=3, name=f"mt_{p}")
        kt_sb, v_sb, lk_sb, ma_sb, mab_sb = {}, {}, {}, {}, {}
        for j, b in enumerate(bs):
            et = []
            for c in range(DC):
                e_c = sb.tile([128, N], BF16, tag="et", bufs=12, name=f"et{b}_{c}")
                nc.sync.dma_start(e_c[:], embT[b, c])
                et.append(e_c)
            for c in range(DC):
                msc = sb.tile([128, N], BF16, tag="msc", bufs=2, name=f"msc{b}{c}")
                nc.scalar.activation(msc[:], et[c][:], AF.Copy,
                                     accum_out=mt[:, 2 * c + j : 2 * c + j + 1])
            for m in range(DC):
                kt_ps = pskv.tile([128, N], F32, tag="kv", name=f"ktps{b}{m}")
                for k in range(DC):
                    nc.tensor.matmul(kt_ps[:], wkv_t[k][:, bass.ts(m, 128)], et[k][:],
                                     start=(k == 0), stop=(k == DC - 1))
                kt = sb.tile([128, N], BF16, tag="kt", bufs=18, name=f"kt{b}_{m}")
                nc.vector.tensor_copy(kt[:], kt_ps[:])
                kt_sb[b, m] = kt
            for m in range(DC):
                v_ps = pskv.tile([128, H], F32, tag="kv", name=f"vps{b}{m}")
                for k in range(DC):
                    nc.tensor.matmul(v_ps[:], et[k][:, bass.ts(m, 128)],
                                     wkv_t[k][:, H : 2 * H],
                                     start=(k == 0), stop=(k == DC - 1))
                v = sb.tile([128, H], BF16, tag="v", bufs=18, name=f"v{b}_{m}")
                nc.vector.tensor_copy(v[:], v_ps[:])
                v_sb[b, m] = v
            for m in range(DC):
                lk_ps = pskv.tile([128, N], F32, tag="kv", name=f"lkps{b}{m}")
                for k in range(DC):
                    nc.tensor.matmul(lk_ps[:], wkv_t[k][:, 2 * H + m * 128 : 2 * H + (m + 1) * 128],
                                     et[k][:], start=(k == 0), stop=(k == DC - 1))
                lk = sb.tile([128, N], BF16, tag="lk", bufs=18, name=f"lk{b}_{m}")
                nc.scalar.copy(lk[:], lk_ps[:])
                lk_sb[b, m] = lk
            ma_t = sb.tile([128, N], F32, tag="ma", bufs=6, name=f"ma{b}")
            nc.sync.dma_start(ma_t[:], ma[b])
            ma_sb[b] = ma_t
            mab_t = sb.tile([128, N], BF16, tag="mab", bufs=6, name=f"mab{b}")
            nc.sync.dma_start(mab_t[:], mab[b])
            mab_sb[b] = mab_t

        # ---- per-pair: fixed context + query ----
        mtb = sb.tile([128, DC * 2], BF16, tag="mtb", bufs=3, name=f"mtb_{p}")
        nc.vector.tensor_copy(mtb[:], mt[:])
        fc_sb = []
        for m in range(DC):
            fc_ps = pssm.tile([128, 2], F32, tag="sm", name=f"fcps{p}{m}")
            for k in range(DC):
                nc.tensor.matmul(fc_ps[:], wctx_t[k][:, bass.ts(m, 128)],
                                 mtb[:, 2 * k : 2 * k + 2],
                                 start=(k == 0), stop=(k == DC - 1))
            fc_m = sb.tile([128, 2], F32, tag="fc", bufs=8, name=f"fc{p}_{m}")
            nc.scalar.activation(fc_m[:], fc_ps[:], AF.Copy, scale=0.125 / N)
            fc_sb.append(fc_m)
        nnt = []
        for k in range(KC):
            nn_k = sb.tile([128, 2 * T], BF16, tag="nnt", bufs=12, name=f"nnt{p}_{k}")
            nc.sync.dma_start(nn_k[:, 0:T], nnT[bs[0], k])
            nc.sync.dma_start(nn_k[:, T : 2 * T], nnT[bs[1], k])
            nnt.append(nn_k)
        qt_sb = []
        for m in range(DC):
            q_ps = pssm.tile([128, 2 * T], F32, tag="sm", name=f"qps{p}{m}")
            for k in range(KC):
                nc.tensor.matmul(q_ps[:], wstep_t[k][:, bass.ts(m, 128)], nnt[k][:],
                                 start=(k == 0), stop=(k == KC - 1))
            qt = sb.tile([128, 2 * T], BF16, tag="qt", bufs=8, name=f"qt{p}_{m}")
            for j in range(2):
                nc.vector.tensor_scalar(qt[:, j * T : (j + 1) * T],
                                        q_ps[:, j * T : (j + 1) * T],
                                        0.125, fc_sb[m][:, j : j + 1],
                                        op0=OP.mult, op1=OP.add)
            qt_sb.append(qt)

        # ---- per-batch: compat + softmax (unnormalized P + diag(1/s)) ----
        P_sb, dg_sb = {}, {}
        for j, b in enumerate(bs):
            for hp in range(HEADS // 2):
                # the two QK matmuls use disjoint PE row groups (partition
                # offsets 0 and 64) and run concurrently when adjacent.
                cms = []
                for hl in range(2):
                    h = 2 * hp + hl
                    cm_ps = pscm.tile([128, N], F32, tag="cm", name=f"cm{b}{h}")
                    nc.tensor.matmul(cm_ps[:], ident[:], mab_sb[b][:],
                                     start=True, stop=False)
                    cms.append(cm_ps)
                for hl in range(2):
                    h = 2 * hp + hl
                    o = hl * 64
                    nc.tensor.matmul(cms[hl][:],
                                     qt_sb[hp][o : o + 64, j * T : (j + 1) * T],
                                     kt_sb[b, hp][o : o + 64, :], start=False, stop=True)
                for hl in range(2):
                    h = 2 * hp + hl
                    pe = sb.tile([128, N], BF16, tag="pe", bufs=22, name=f"pe{b}{h}")
                    s = sb.tile([128, 1], F32, tag="s", bufs=22, name=f"s{b}{h}")
                    nc.scalar.activation(pe[:], cms[hl][:], AF.Exp, accum_out=s[:])
                    r = sb.tile([128, 1], F32, tag="r", bufs=22, name=f"r{b}{h}")
                    nc.vector.reciprocal(r[:], s[:])
                    dg = sb.tile([128, 128], BF16, tag="dg", bufs=22, name=f"dg{b}{h}")
                    nc.vector.tensor_scalar_mul(dg[:], ident[:], r[:])
                    P_sb[b, h] = pe
                    dg_sb[b, h] = dg

        # ---- per-pair: transpose attn, heads_out ----
        hd_sb = [sb.tile([128, 2 * T], BF16, tag="hd", bufs=6, name=f"hd{p}_{m}")
                 for m in range(DC)]
        for hp in range(HEADS // 2):
            atts = {}
            for hl in range(2):
                h = 2 * hp + hl
                att2 = []
                for q in range(DC // 2):
                    at_ps = pssm.tile([128, 4 * T], F32, tag="sm", name=f"at{p}{h}{q}")
                    for cl in range(2):
                        for j, b in enumerate(bs):
                            nc.tensor.matmul(at_ps[:, (cl * 2 + j) * T : (cl * 2 + j + 1) * T],
                                             P_sb[b, h][:, bass.ts(2 * q + cl, 128)],
                                             dg_sb[b, h][:], start=True, stop=True)
                    a_q = sb.tile([128, 4 * T], BF16, tag="att", bufs=8, name=f"att{p}{h}{q}")
                    nc.vector.tensor_copy(a_q[:], at_ps[:])
                    att2.append(a_q)
                atts[hl] = att2
            # the pair's two heads accumulate in separate banks but
            # disjoint PE col groups (partitions 0-63 vs 64-127), so the
            # interleaved chains overlap on the array.
            hd_a = pssm.tile([64, 2 * T], F32, tag="sm", name=f"hdpa{p}{hp}")
            hd_b = pssm.tile([128, 2 * T], F32, tag="sm", name=f"hdpb{p}{hp}")
            for j, b in enumerate(bs):
                for c in range(DC):
                    for hl in range(2):
                        h = 2 * hp + hl
                        o = hl * 64
                        if hl == 0:
                            out = hd_a[:, j * T : (j + 1) * T]
                        else:
                            out = hd_b[64:128, j * T : (j + 1) * T]
                        nc.tensor.matmul(out,
                                         v_sb[b, c][:, h * 64 : (h + 1) * 64],
                                         atts[hl][c // 2][:, ((c % 2) * 2 + j) * T : ((c % 2) * 2 + j + 1) * T],
                                         start=(c == 0), stop=(c == DC - 1),
                                         tile_position=(0, o))
            nc.vector.tensor_copy(hd_sb[hp][0:64, :], hd_a[:])
            nc.vector.tensor_copy(hd_sb[hp][64:128, :], hd_b[64:128, :])

        # ---- per-pair: glimpse ----
        g_sb = []
        for m in range(DC):
            g_ps = pssm.tile([128, 2 * T], F32, tag="sm", name=f"gps{p}{m}")
            for k in range(DC):
                nc.tensor.matmul(g_ps[:], wout_t[k][:, bass.ts(m, 128)], hd_sb[k][:],
                                 start=(k == 0), stop=(k == DC - 1))
            g_m = sb.tile([128, 2 * T], BF16, tag="g", bufs=6, name=f"g{p}_{m}")
            nc.vector.tensor_copy(g_m[:], g_ps[:])
            g_sb.append(g_m)

        # ---- per-batch: logits + log_softmax ----
        for j, b in enumerate(bs):
            lg_ps = pscm.tile([128, N], F32, tag="cm", name=f"lg{b}")
            for k in range(DC):
                nc.tensor.matmul(lg_ps[:], g_sb[k][:, j * T : (j + 1) * T],
                                 lk_sb[b, k][:], start=(k == 0), stop=(k == DC - 1))
            y = sb.tile([128, N], F32, tag="y", bufs=3, name=f"y{b}")
            nc.scalar.activation(y[:], lg_ps[:], AF.Tanh, scale=float(1.0 / np.sqrt(H)))
            t2 = sb.tile([128, N], F32, tag="t2", bufs=3, name=f"t2{b}")
            nc.vector.tensor_tensor(t2[:], y[:], ma_sb[b][:], op=OP.add)
            p2 = sb.tile([128, N], BF16, tag="p2", bufs=2, name=f"p2{b}")
            s2 = sb.tile([128, 1], F32, tag="s2", bufs=4, name=f"s2{b}")
            nc.scalar.activation(p2[:], t2[:], AF.Exp, scale=10.0, accum_out=s2[:])
            # ln(s2) without the ACT Ln table: Mitchell bit-trick seed
            # y0 = (int_view(s2) * ln2/2^23) - (127 - 0.0430)*ln2  (|err|<=0.03)
            # then 2 Newton steps  y <- y + s2*exp(-y) - 1.
            LN2 = float(np.log(2.0))
            vi = sb.tile([128, 1], F32, tag="vi", bufs=4, name=f"vi{b}")
            nc.vector.tensor_copy(vi[:], s2[:].bitcast(mybir.dt.int32))
            y0 = sb.tile([128, 1], F32, tag="lns", bufs=4, name=f"lns{b}")
            nc.vector.tensor_scalar(y0[:], vi[:], LN2 / (1 << 23),
                                    (127.0 - 0.0430) * LN2, op0=OP.mult, op1=OP.subtract)
            lns = y0
            for it in range(2):
                ex = sb.tile([128, 1], F32, tag="nex", bufs=8, name=f"nex{b}{it}")
                nc.scalar.activation(ex[:], lns[:], AF.Exp, scale=-1.0)
                dl = sb.tile([128, 1], F32, tag="ndl", bufs=8, name=f"ndl{b}{it}")
                nc.vector.tensor_scalar(dl[:], ex[:], s2[:], 1.0,
                                        op0=OP.mult, op1=OP.subtract)
                ln2t = sb.tile([128, 1], F32, tag="lns", bufs=4, name=f"lns{b}_{it}")
                nc.vector.tensor_tensor(ln2t[:], lns[:], dl[:], op=OP.add)
                lns = ln2t
            o_t = sb.tile([128, N], F32, tag="o", bufs=3, name=f"o{b}")
            nc.vector.tensor_scalar(o_t[:], t2[:], 10.0, lns[:], op0=OP.mult, op1=OP.subtract)
            nc.sync.dma_start(outp[b], o_t[:])


def _build(bl, reps=1):
    nc = bacc.Bacc("TRN2", target_bir_lowering=False, debug=False)
    embT = nc.dram_tensor("embT", [bl, DC, 128, N], BF16, kind="ExternalInput").ap()
    nnT = nc.dram_tensor("nnT", [bl, KC, 128, T], BF16, kind="ExternalInput").ap()
    ma = nc.dram_tensor("ma", [bl, T, N], F32, kind="ExternalInput").ap()
    mab = nc.dram_tensor("mab", [bl, T, N], BF16, kind="ExternalInput").ap()
    wkv = nc.dram_tensor("wkv", [DC, 128, 3 * H], BF16, kind="ExternalInput").ap()
    wstep = nc.dram_tensor("wstep", [KC, 128, H], BF16, kind="ExternalInput").ap()
    wout = nc.dram_tensor("wout", [DC, 128, H], BF16, kind="ExternalInput").ap()
    wctx = nc.dram_tensor("wctx", [DC, 128, H], BF16, kind="ExternalInput").ap()
    outp = nc.dram_tensor("logp", [bl, T, N], F32, kind="ExternalOutput").ap()
    with tile.TileContext(nc) as tc:
        for _ in range(reps):
            with ExitStack() as ctx:
                _emit(ctx, tc, (embT, nnT, ma, mab, wkv, wstep, wout, wctx, outp), bl)
    nc.compile()
    return nc


_cache = {}


def _program(bl, reps=1):
    if (bl, reps) not in _cache:
        _cache[(bl, reps)] = _build(bl, reps)
    return _cache[(bl, reps)]


def _prep(embedding, current_nodes, used_capacity, used_battery, current_time, mask):
    bf = ml_dtypes.bfloat16
    b = embedding.shape[0]
    embT = np.ascontiguousarray(embedding.transpose(0, 2, 1)).astype(bf)
    embT = embT.reshape(b, DC, 128, N)
    cur = np.take_along_axis(embedding, current_nodes.astype(np.int64)[:, :, None], axis=1)
    nnT = np.zeros((b, KC * 128, T), np.float32)
    nnT[:, :D, :] = cur.transpose(0, 2, 1)
    nnT[:, D, :] = 1.0 - used_capacity
    nnT[:, D + 1, :] = 1.0 - used_battery
    nnT[:, D + 2, :] = current_time
    nnT = nnT.astype(bf).reshape(b, KC, 128, T)
    ma = np.where(mask, np.float32(MA), np.float32(0.0))
    mab = ma.astype(ml_dtypes.bfloat16)
    return embT, nnT, ma, mab


def _prep_weights(W_context, W_kvlogit, W_step, W_out):
    bf = ml_dtypes.bfloat16
    wkv = W_kvlogit.astype(bf).reshape(DC, 128, 3 * H)
    ws = np.zeros((KC * 128, H), np.float32)
    ws[: D + 3] = W_step
    wstep = ws.astype(bf).reshape(KC, 128, H)
    wout = W_out.astype(bf).reshape(DC, 128, H)
    wctx = W_context.astype(bf).reshape(DC, 128, H)
    return wkv, wstep, wout, wctx


def kernel(embedding, current_nodes, used_capacity, used_battery, current_time,
           mask, W_context, W_kvlogit, W_step, W_out):
    global LAST_EXEC_TIME_NS
    embedding = np.asarray(embedding, np.float32)
    mask = np.asarray(mask, bool)
    embT, nnT, ma, mab = _prep(np.asarray(embedding, np.float32),
                          np.asarray(current_nodes),
                          np.asarray(used_capacity, np.float32),
                          np.asarray(used_battery, np.float32),
                          np.asarray(current_time, np.float32), mask)
    wkv, wstep, wout, wctx = _prep_weights(
        np.asarray(W_context, np.float32), np.asarray(W_kvlogit, np.float32),
        np.asarray(W_step, np.float32), np.asarray(W_out, np.float32))
    nc = _program(BL)
    in_maps = []
    for c in range(NCORES):
        s = slice(c * BL, (c + 1) * BL)
        in_maps.append({"embT": embT[s], "nnT": nnT[s], "ma": ma[s], "mab": mab[s],
                        "wkv": wkv, "wstep": wstep, "wout": wout, "wctx": wctx})
    res = run_bass_kernel_spmd(nc, in_maps, list(range(NCORES)))
    LAST_EXEC_TIME_NS = res.exec_time_ns
    return np.concatenate([res.results[c]["logp"] for c in range(NCORES)], axis=0)

